# revision 11
# baseline (speedup 1.0000x reference)
"""Causal self-attention (B=4, T=2048, D=1024, H=16) on 8 TRN2 NeuronCores.

Sharding: core i = (batch b = i//2, head-group g = i%2). Data parallel on B,
tensor parallel on heads (8 heads per group): qkv_proj columns and out_proj
rows split per head group. Each core computes a partial [D, T] output^T for
its batch; host sums the two group partials per batch, transposes, adds bias.

v2 changes vs baseline (396-405us):
  - exp split across Scalar (ACT) + a custom Vector (DVE) limit-exp
    (1 + u/8192)^8192 done as (1+u*2^-16)^64 then ^128, causal mask fused
    into the DVE pass-2; ACT-assigned diagonal strips get a gpsimd mask-mul.
    Removes the ACT bottleneck in phase 2 (853ns/j-step exp vs 640ns PE).
  - per-(pair,qc) Q/K chunk prefetch 2 steps ahead (starts during phase 1):
    kills the 2.7us pair-boundary PE gaps that re-throttled HAM to 1.2GHz
    (~94us of half-clock in the baseline trace).
  - x loaded one [128, C] DMA per 128-token block instead of 8 [128,128].
  - softmax scale folded into the exp (ACT scale=0.125 / DVE imm), the
    Q-bounce scalar.mul becomes a plain copy split across ACT/DVE.
  - V, P (attn weights), O~, Wo in bf16 (S stays f32r); normalize fused:
    reciprocal reads PSUM directly, one tensor_mul does PSUM->OT.
"""

import numpy as np

import concourse.bacc as bacc
import concourse.tile as tile
import concourse.mybir as mybir
from concourse import bass_utils
from concourse.bass import ts

F32 = mybir.dt.float32
F32R = mybir.dt.float32r
BF16 = mybir.dt.bfloat16
EXP = mybir.ActivationFunctionType.Exp

T = 2048
TT = 16          # t tiles of 128
NP = 4           # head pairs per core
NQC = 4          # q chunks of 512
SCALE = 0.125    # 1/sqrt(64)
EXP_N = 8192     # limit-exp exponent: (1 + u/EXP_N)^EXP_N ~= exp(u)

_CACHE = {}
_last_in_maps = None

# ---------------------------------------------------------------------------
# custom DVE exp:  pass1  t = (1 + x*imm)^64        (mul, add, 6 squarings)
#                  pass2  p = t^128                 (7 squarings)
#                  pass2m p = t^128 * mask          (7 squarings + mul)
# total (1 + u/8192)^8192, u = SCALE*S; imm = SCALE/8192 = 2^-16 exact.
_EXP_OPS = None


def _ensure_exp_ops():
    global _EXP_OPS
    if _EXP_OPS is not None:
        return _EXP_OPS
    from concourse import dve_ops as dops
    from concourse.dve_spec import Spec, Src0, Src1, One, C2, lower
    from concourse.dve_spec import _has_src1 as has_src1
    from concourse.dve_uop import DveOpSpec
    from concourse.dve_table_gen import dve_ver_for

    ver = dve_ver_for("TRN2")
    existing = {o.name: o for o in dops.OPS}

    def mk(name, spec):
        if name in existing:
            return existing[name]
        row = max(dops._SUB_OPCODE_FOR_NAME.values()) + 1
        assert row < 0x20, "custom-DVE opcode rows exhausted"
        dops._SUB_OPCODE_FOR_NAME[name] = row
        s = DveOpSpec(
            name=name, opcode=row, uops=lower(spec, ver=ver),
            rd1_en=has_src1(spec),
        )
        op = dops.DveOp(name, spec, subdim=False, uops_sha={ver: s.sha(ver)})
        dops.OPS.append(op)
        dops.CUSTOM_DVE_SPECS[name] = spec
        return op

    p = One + Src0 * C2
    for _ in range(6):
        p = p * p
    q = Src0
    for _ in range(7):
        q = q * q
    qm = Src0
    for _ in range(7):
        qm = qm * qm
    _EXP_OPS = (
        mk("ANT_EXP_P1", Spec(body=p)),
        mk("ANT_EXP_SQ7", Spec(body=q)),
        mk("ANT_EXP_SQ7M", Spec(body=qm * Src1)),
    )
    return _EXP_OPS


def _build(CT):
    """CT = number of 128-row c-tiles in the (possibly bias-augmented) x/W."""
    EXP_P1, EXP_SQ7, EXP_SQ7M = _ensure_exp_ops()
    nc = bacc.Bacc("TRN2", target_bir_lowering=False, debug=False)
    C = CT * 128

    xa = nc.dram_tensor("xa", [T, C], F32, kind="ExternalInput").ap()
    wq = nc.dram_tensor("wq", [C, 512], F32, kind="ExternalInput").ap()
    wk = nc.dram_tensor("wk", [C, 512], F32, kind="ExternalInput").ap()
    wv = nc.dram_tensor("wv", [C, 512], F32, kind="ExternalInput").ap()
    wo = nc.dram_tensor("wo", [512, 1024], F32, kind="ExternalInput").ap()
    tri = nc.dram_tensor("tri", [128, 128], F32, kind="ExternalInput").ap()
    idn = nc.dram_tensor("idn", [128, 128], F32, kind="ExternalInput").ap()
    ot = nc.dram_tensor("ot", [1024, T], F32, kind="ExternalOutput").ap()

    mm = nc.tensor.matmul
    mmt = nc.tensor.transpose

    with tile.TileContext(nc) as tc:
        with (
            tc.tile_pool(name="persist", bufs=1) as persist,
            tc.tile_pool(name="dram", bufs=1, space="DRAM") as dpool,
            tc.tile_pool(name="kch", bufs=6) as kpool,
            tc.tile_pool(name="qch", bufs=4) as qpool,
        ):
            vS = persist.tile([128, TT, 8, 65], BF16)     # [k128, ktile, head, d+1]
            OT = persist.tile([128, NP, T], BF16)         # [d128(2 heads), pair, t]
            tr = persist.tile([128, 128], F32)            # keep-mask f32
            tr16 = persist.tile([128, 128], BF16)         # keep-mask bf16
            wo_sb = persist.tile([128, NP, 1024], BF16)
            nc.vector.memset(vS[:, :, :, 64:65], 1.0)

            # Q^T / K^T bounce chunks, one DRAM tile per (pair, t-chunk)
            qtd = {}
            ktd = {}
            for _p in range(NP):
                for _tc in range(4):
                    qtd[(_p, _tc)] = dpool.tile([128, 512], F32, name=f"qtd{_p}{_tc}")
                    ktd[(_p, _tc)] = dpool.tile([128, 512], F32, name=f"ktd{_p}{_tc}")

            # phase-2 chunk fetch (emitted partly during phase 1) ----------
            qch = {}
            kch = {}
            fetch_seq = [(p, qc) for p in range(NP) for qc in range(NQC)]
            fetched = [0]  # count of fetch_seq entries already fetched

            def fetch_upto(n):
                while fetched[0] < min(n, len(fetch_seq)):
                    p_, qc_ = fetch_seq[fetched[0]]
                    kt = kpool.tile([128, 512], F32R, name="kTc", tag="kTc")
                    nc.gpsimd.dma_start(out=kt, in_=ktd[(p_, qc_)].bitcast(F32R))
                    kch[(p_, qc_)] = kt
                    qt = qpool.tile([128, 512], F32R, name="qTc", tag="qTc")
                    nc.gpsimd.dma_start(out=qt, in_=qtd[(p_, qc_)].bitcast(F32R))
                    qch[(p_, qc_)] = qt
                    fetched[0] += 1

            # ---------------- phase 1: transpose + projections ----------------
            with (
                tc.tile_pool(name="ph1", bufs=1) as ph1,
                tc.tile_pool(name="xnat", bufs=3) as xnat,
                tc.tile_pool(name="bounce", bufs=2) as bpool,
                tc.tile_pool(name="pst", bufs=2, space="PSUM") as pst,
                tc.tile_pool(name="psp", bufs=6, space="PSUM") as psp,
            ):
                ident = ph1.tile([128, 128], F32)
                nc.sync.dma_start(out=ident, in_=idn)
                wv_sb = ph1.tile([128, CT, 512], F32R)
                wq_sb = ph1.tile([128, CT, NP, 128], F32R)
                wk_sb = ph1.tile([128, CT, NP, 128], F32R)
                xT = [ph1.tile([128, T], F32R, name=f"xT{cc}") for cc in range(CT)]
                cp_flip = [0]

                def copy_alt(out, in_):
                    # alternate PSUM-evacuation copies between DVE and ACT
                    eng = nc.vector if cp_flip[0] % 2 == 0 else nc.scalar
                    cp_flip[0] += 1
                    if eng is nc.vector:
                        eng.tensor_copy(out=out, in_=in_)
                    else:
                        eng.copy(out=out, in_=in_)

                def load_weights():
                    nc.gpsimd.dma_start(out=tr, in_=tri)
                    nc.gpsimd.dma_start(
                        out=wv_sb,
                        in_=wv.rearrange("(ct P) f -> P ct f", P=128).bitcast(F32R),
                    )
                    nc.gpsimd.dma_start(
                        out=wq_sb,
                        in_=wq.rearrange("(ct P) (np f) -> P ct np f", P=128, np=NP).bitcast(F32R),
                    )
                    nc.gpsimd.dma_start(
                        out=wk_sb,
                        in_=wk.rearrange("(ct P) (np f) -> P ct np f", P=128, np=NP).bitcast(F32R),
                    )
                    nc.gpsimd.tensor_copy(out=tr16, in_=tr)

                def load_wo():
                    wo_f = ph1.tile([128, NP, 1024], F32)
                    nc.gpsimd.dma_start(
                        out=wo_f, in_=wo.rearrange("(np P) f -> P np f", P=128)
                    )
                    nc.gpsimd.tensor_copy(out=wo_sb, in_=wo_f)

                def transpose_tt(tt):
                    xn = xnat.tile([128, C], F32)
                    nc.sync.dma_start(out=xn, in_=xa[ts(tt, 128), :])
                    for cc in range(CT):
                        pt_ = pst.tile([128, 512], F32)
                        mmt(pt_[:, :128], xn[:, ts(cc, 128)], ident)
                        copy_alt(xT[cc][:, ts(tt, 128)], pt_[:, :128])

                def vproj_tt(tt):
                    ps = psp.tile([128, 512], F32)
                    for cc in range(CT):
                        mm(
                            ps,
                            lhsT=xT[cc][:, ts(tt, 128)],
                            rhs=wv_sb[:, cc, :],
                            start=(cc == 0),
                            stop=(cc == CT - 1),
                        )
                    nc.vector.tensor_copy(
                        out=vS[:, tt, :, 0:64],
                        in_=ps.rearrange("p (h d) -> p h d", h=8),
                    )

                def qkproj_tc(tc_):
                    for p in range(NP):
                        for w_sb, dst in ((wq_sb, qtd), (wk_sb, ktd)):
                            ps = psp.tile([128, 512], F32)
                            for cc in range(CT):
                                mm(
                                    ps,
                                    lhsT=w_sb[:, cc, p, :],
                                    rhs=xT[cc][:, ts(tc_, 512)],
                                    start=(cc == 0),
                                    stop=(cc == CT - 1),
                                )
                            bo = bpool.tile([128, 512], F32)
                            copy_alt(bo, ps)
                            nc.scalar.dma_start(out=dst[(p, tc_)], in_=bo)

                for tt in range(TT + 1):
                    if tt < TT:
                        transpose_tt(tt)
                    if tt == 0:
                        load_weights()
                    if tt == 2:
                        load_wo()
                    if tt >= 1:
                        vproj_tt(tt - 1)
                        if (tt - 1) % 4 == 3:
                            qkproj_tc((tt - 1) // 4)
                            # prefetch phase-2 chunks as soon as they land
                            fetch_upto((tt - 1) // 4 + 1)

            # ---------------- phase 2: attention ----------------
            # Emission keeps the PE in same-type runs: a group of 3 j-steps of
            # S matmuls, then the previous group's AV matmuls (LAG=2 halves).
            # exp of each strip goes to ACT or DVE by greedy load balance.
            with (
                tc.tile_pool(name="ptp", bufs=12) as ptpool,
                tc.tile_pool(name="tmp", bufs=6) as tmppool,
                tc.tile_pool(name="rsm", bufs=4) as rpool,
                tc.tile_pool(name="rbcp", bufs=4) as rbcpool,
                tc.tile_pool(name="psS", bufs=3, space="PSUM") as psS,
                tc.tile_pool(name="psAv", bufs=2, space="PSUM") as psAv,
            ):
                avs = {}
                pts = {}
                eng_load = [0.0, 0.0]  # ns accumulated: [ACT, DVE]
                trb16 = tr16[:, None, :].to_broadcast([128, 2, 128])
                trbf = tr[:, None, :].to_broadcast([128, 2, 128])

                def s_exp(p, qc, j):
                    off = max(0, 128 * j - 512 * qc)
                    sg = psS.tile([128, 2, 512], F32)
                    kc = kch[(p, j // 4)]
                    qc_t = qch[(p, qc)]
                    jo = 128 * (j % 4)
                    for m in range(2):
                        mm(
                            sg[:, m, off:],
                            lhsT=kc[64 * m : 64 * m + 64, jo : jo + 128],
                            rhs=qc_t[64 * m : 64 * m + 64, off:],
                            start=True,
                            stop=True,
                        )
                    ptile = ptpool.tile([128, 2, 512], BF16)
                    masked = j >= 4 * qc
                    cols = 2 * (512 - off)
                    cost_a = cols * 0.95 + 150.0
                    cost_d = cols * 2.084 + 180.0
                    use_act = eng_load[0] + cost_a <= eng_load[1] + cost_d
                    if use_act:
                        eng_load[0] += cost_a
                        nc.scalar.activation(
                            out=ptile[:, :, off:], in_=sg[:, :, off:],
                            func=EXP, scale=SCALE,
                        )
                        if masked:
                            nc.gpsimd.tensor_mul(
                                ptile[:, :, off : off + 128],
                                ptile[:, :, off : off + 128],
                                trb16,
                            )
                    else:
                        eng_load[1] += cost_d
                        tmp = tmppool.tile([128, 2, 512], F32)
                        nc.vector._custom_dve(
                            EXP_P1, out=tmp[:, :, off:], in0=sg[:, :, off:],
                            imm2=SCALE / EXP_N,
                        )
                        if masked:
                            nc.vector._custom_dve(
                                EXP_SQ7M,
                                out=ptile[:, :, off : off + 128],
                                in0=tmp[:, :, off : off + 128],
                                in1=trbf,
                            )
                            if off + 128 < 512:
                                nc.vector._custom_dve(
                                    EXP_SQ7,
                                    out=ptile[:, :, off + 128 :],
                                    in0=tmp[:, :, off + 128 :],
                                )
                        else:
                            nc.vector._custom_dve(
                                EXP_SQ7, out=ptile[:, :, off:], in0=tmp[:, :, off:]
                            )
                    pts[(p, qc, j)] = (ptile, off)

                def av_mm(p, qc, j, nj):
                    ptile, off = pts.pop((p, qc, j))
                    av = avs[(p, qc)]
                    for m in range(2):
                        mm(
                            av[m][:65, off:],
                            lhsT=vS[:, j, 2 * p + m, :],
                            rhs=ptile[:, m, off:],
                            start=(j == 0),
                            stop=(j == nj - 1),
                        )

                def normalize(p, qc):
                    av = avs.pop((p, qc))
                    rinvs = []
                    for m in range(2):
                        # sigma lives at psum partition 64: move to partition 0
                        # with a plain copy (per-operand partition shifts are
                        # only safe on tensor_copy), then reciprocal aligned.
                        rsb = rpool.tile([1, 512], F32, name="rsb", tag="rsb")
                        nc.vector.tensor_copy(out=rsb, in_=av[m][64:65, :])
                        rinv = rpool.tile([1, 512], F32, name="rinv", tag="rinv")
                        nc.vector.reciprocal_approx_fast(out=rinv, in_=rsb)
                        rinvs.append(rinv)
                    for m in range(2):
                        rb = rbcpool.tile([128, 512], F32, name="rb", tag="rb")
                        nc.gpsimd.partition_broadcast(rb, rinvs[m])
                        dst = OT[64 * m : 64 * m + 64, p, ts(qc, 512)]
                        if m == 0:
                            # all operands base-0 aligned: fuse PSUM->SBUF
                            # copy with the normalize multiply
                            nc.vector.tensor_mul(dst, av[m][0:64, :], rb[0:64, :])
                            eng_load[1] += 660.0
                        else:
                            nc.vector.tensor_copy(out=dst, in_=av[m][0:64, :])
                            nc.gpsimd.tensor_mul(dst, dst, rb[64:128, :])
                            eng_load[1] += 660.0

                groups = []
                for p in range(NP):
                    for qc in range(NQC):
                        nj = 4 * qc + 4
                        js = list(range(nj))
                        sub = [js[i : i + 3] for i in range(0, nj, 3)]
                        for gi, jg in enumerate(sub):
                            groups.append((p, qc, nj, jg, gi == 0, gi == len(sub) - 1))

                def av_group(gi):
                    p, qc, nj, jg, first, last = groups[gi]
                    if first:
                        avs[(p, qc)] = [
                            psAv.tile([128, 512], F32, name="av", tag="av")
                            for _ in range(2)
                        ]
                    for j in jg:
                        av_mm(p, qc, j, nj)
                    if last:
                        normalize(p, qc)

                LAG = 2
                seq_pos = {}
                for i, (p_, qc_) in enumerate(fetch_seq):
                    seq_pos[(p_, qc_)] = i
                for i in range(len(groups) + LAG):
                    if i < len(groups):
                        p, qc, nj, jg, first, last = groups[i]
                        if first:
                            # keep the chunk fetches 2 (p,qc)-steps ahead
                            fetch_upto(seq_pos[(p, qc)] + 3)
                        for j in jg:
                            s_exp(p, qc, j)
                    if i >= LAG and (i - LAG) % 2 == 1:
                        av_group(i - LAG - 1)
                        av_group(i - LAG)
                if len(groups) % 2 == 1:
                    av_group(len(groups) - 1)

            # ---------------- phase 3: output projection ----------------
            with (
                tc.tile_pool(name="obnc", bufs=4) as opool,
                tc.tile_pool(name="psO", bufs=8, space="PSUM") as psO,
            ):
                ob_flip = [0]
                for tc_ in range(4):
                    for ft in range(8):
                        pso = psO.tile([128, 512], F32, name="pso", tag="pso")
                        for p in range(NP):
                            mm(
                                pso,
                                lhsT=wo_sb[:, p, ts(ft, 128)],
                                rhs=OT[:, p, ts(tc_, 512)],
                                start=(p == 0),
                                stop=(p == NP - 1),
                            )
                        ob = opool.tile([128, 512], F32)
                        if ob_flip[0] % 2 == 0:
                            nc.vector.tensor_copy(out=ob, in_=pso)
                        else:
                            nc.scalar.copy(out=ob, in_=pso)
                        ob_flip[0] += 1
                        nc.sync.dma_start(out=ot[ts(ft, 128), ts(tc_, 512)], in_=ob)

    nc.compile()
    return nc


def kernel(x, W_qkv, b_qkv, W_out, b_out):
    global _last_in_maps
    x = np.asarray(x, dtype=np.float32)
    W_qkv = np.asarray(W_qkv, dtype=np.float32)
    b_qkv = np.asarray(b_qkv, dtype=np.float32)
    W_out = np.asarray(W_out, dtype=np.float32)
    b_out = np.asarray(b_out, dtype=np.float32)
    B = x.shape[0]

    aug = bool(np.any(b_qkv))
    CT = 9 if aug else 8
    if CT not in _CACHE:
        _CACHE[CT] = _build(CT)
    nc = _CACHE[CT]

    # triangle keep-mask for the diagonal 128 block: [p, c] = 1 if c >= p
    tri = (np.arange(128)[None, :] >= np.arange(128)[:, None]).astype(np.float32)

    in_maps = []
    for core in range(8):
        b, g = core // 2, core % 2
        xa = x[b]
        if aug:
            pad = np.zeros((T, 128), np.float32)
            pad[:, 0] = 1.0
            xa = np.concatenate([xa, pad], axis=1)

        def wslice(col0):
            w = W_qkv[:, col0 + 512 * g : col0 + 512 * g + 512]
            if aug:
                extra = np.zeros((128, 512), np.float32)
                extra[0] = b_qkv[col0 + 512 * g : col0 + 512 * g + 512]
                w = np.concatenate([w, extra], axis=0)
            return np.ascontiguousarray(w)

        in_maps.append(
            {
                "xa": np.ascontiguousarray(xa),
                "wq": wslice(0),
                "wk": wslice(1024),
                "wv": wslice(2048),
                "wo": np.ascontiguousarray(W_out[512 * g : 512 * g + 512, :]),
                "tri": tri,
                "idn": np.eye(128, dtype=np.float32),
            }
        )

    _last_in_maps = in_maps
    res = bass_utils.run_bass_kernel_spmd(nc, in_maps, list(range(8))).results
    out = np.empty((B, T, 1024), np.float32)
    for b in range(B):
        acc = res[2 * b]["ot"] + res[2 * b + 1]["ot"]
        out[b] = acc.T + b_out[None, :]
    return out


# revision 12
# speedup vs baseline: 1.6072x; 1.6072x over previous
"""Causal self-attention (B=4, T=2048, D=1024, H=16) on 8 TRN2 NeuronCores.

Sharding: core i = (batch b = i//2, head-group g = i%2). Data parallel on B,
tensor parallel on heads (8 heads per group): qkv_proj columns and out_proj
rows split per head group. Each core computes a partial [D, T] output^T for
its batch; host sums the two group partials per batch, transposes, adds bias.

v3 design (baseline 396-405us):
  - x is transposed on the HOST (xat = x[b].T): no PE transposes at all.
    Phase 1 streams xT chunks in 512-col slices and runs V/Q/K projections.
  - softmax runs in the log2 domain: W_q is host-folded with 0.125*log2(e),
    so S = log2-weights. ACT exp uses scale=ln2; the custom DVE limit-exp
    computes (1 + y*ln2/8192)^8192 in two passes (8+7 squarings), causal
    mask fused into the DVE pass-2 where assigned.
  - exp strips are split ACT/DVE by greedy load balance (ACT ~1.3ns/col,
    DVE 2-pass ~1.9ns/col measured). gpsimd does ONLY partition_broadcast
    (+DMA issue): mixing op types thrashes its ucode library (~7us/swap).
  - Q/K bounce, S matmul operands, P, V, O~, Wo are bf16. bf16 K=64 S
    matmuls row-pack the two heads concurrently (f32r would need 2x rows).
  - per-(pair,qc) Q/K chunk prefetch 2 steps ahead (starting in phase 1)
    keeps the PE continuously fed (no HAM re-throttle).
"""

import numpy as np

import concourse.bacc as bacc
import concourse.tile as tile
import concourse.mybir as mybir
from concourse import bass_utils
from concourse.bass import ts

F32 = mybir.dt.float32
F32R = mybir.dt.float32r
BF16 = mybir.dt.bfloat16
EXP = mybir.ActivationFunctionType.Exp

T = 2048
TT = 16          # t tiles of 128
NP = 4           # head pairs per core
NQC = 4          # q chunks of 512
SCALE = 0.125    # 1/sqrt(64)
LOG2E = float(np.log2(np.e))
LN2 = float(np.log(2.0))
EXP_N = 8192     # limit-exp exponent: (1 + u/EXP_N)^EXP_N ~= exp(u)

_CACHE = {}
_last_in_maps = None

# ---------------------------------------------------------------------------
# custom DVE exp:  pass1  t = (1 + y*imm)^64        (mul, add, 6 squarings)
#                  pass2  p = t^128                 (7 squarings)
#                  pass2m p = t^128 * mask          (7 squarings + mul)
# with imm = ln2/8192 this computes ~2^y for log2-domain scores y.
_EXP_OPS = None


def _ensure_exp_ops():
    global _EXP_OPS
    if _EXP_OPS is not None:
        return _EXP_OPS
    from concourse import dve_ops as dops
    from concourse.dve_spec import Spec, Src0, Src1, One, C2, lower
    from concourse.dve_spec import _has_src1 as has_src1
    from concourse.dve_uop import DveOpSpec
    from concourse.dve_table_gen import dve_ver_for

    ver = dve_ver_for("TRN2")
    existing = {o.name: o for o in dops.OPS}

    def mk(name, spec):
        if name in existing:
            return existing[name]
        row = max(dops._SUB_OPCODE_FOR_NAME.values()) + 1
        assert row < 0x20, "custom-DVE opcode rows exhausted"
        dops._SUB_OPCODE_FOR_NAME[name] = row
        s = DveOpSpec(
            name=name, opcode=row, uops=lower(spec, ver=ver),
            rd1_en=has_src1(spec),
        )
        op = dops.DveOp(name, spec, subdim=False, uops_sha={ver: s.sha(ver)})
        dops.OPS.append(op)
        dops.CUSTOM_DVE_SPECS[name] = spec
        return op

    p = One + Src0 * C2
    for _ in range(6):
        p = p * p
    q = Src0
    for _ in range(7):
        q = q * q
    qm = Src0
    for _ in range(7):
        qm = qm * qm
    _EXP_OPS = (
        mk("ANT_EXP_P1", Spec(body=p)),
        mk("ANT_EXP_SQ7", Spec(body=q)),
        mk("ANT_EXP_SQ7M", Spec(body=qm * Src1)),
    )
    return _EXP_OPS


def _build(CT):
    """CT = number of 128-row c-tiles in the (possibly bias-augmented) x/W."""
    EXP_P1, EXP_SQ7, EXP_SQ7M = _ensure_exp_ops()
    nc = bacc.Bacc("TRN2", target_bir_lowering=False, debug=False)
    C = CT * 128

    xat = nc.dram_tensor("xat", [C, T], F32, kind="ExternalInput").ap()
    wq = nc.dram_tensor("wq", [C, 512], F32, kind="ExternalInput").ap()
    wk = nc.dram_tensor("wk", [C, 512], F32, kind="ExternalInput").ap()
    wv = nc.dram_tensor("wv", [C, 512], F32, kind="ExternalInput").ap()
    wo = nc.dram_tensor("wo", [512, 1024], F32, kind="ExternalInput").ap()
    tri = nc.dram_tensor("tri", [128, 128], F32, kind="ExternalInput").ap()
    ot = nc.dram_tensor("ot", [1024, T], F32, kind="ExternalOutput").ap()

    mm = nc.tensor.matmul

    with tile.TileContext(nc) as tc:
        with (
            tc.tile_pool(name="persist", bufs=1) as persist,
            tc.tile_pool(name="dram", bufs=1, space="DRAM") as dpool,
            tc.tile_pool(name="kch", bufs=6) as kpool,
            tc.tile_pool(name="qch", bufs=4) as qpool,
        ):
            vS = persist.tile([128, TT, 8, 65], BF16)     # [k128, ktile, head, d+1]
            OT = persist.tile([128, NP, T], BF16)         # [d128(2 heads), pair, t]
            tr = persist.tile([128, 128], F32)            # keep-mask f32
            tr16 = persist.tile([128, 128], BF16)         # keep-mask bf16
            wo_sb = persist.tile([128, NP, 1024], BF16)
            nc.vector.memset(vS[:, :, :, 64:65], 1.0)

            # Q^T / K^T bounce chunks (bf16), one DRAM tile per (pair, t-chunk)
            qtd = {}
            ktd = {}
            for _p in range(NP):
                for _tc in range(4):
                    qtd[(_p, _tc)] = dpool.tile([128, 512], BF16, name=f"qtd{_p}{_tc}")
                    ktd[(_p, _tc)] = dpool.tile([128, 512], BF16, name=f"ktd{_p}{_tc}")

            qch = {}
            kch = {}
            fetch_seq = [(p, qc) for p in range(NP) for qc in range(NQC)]
            fetched = [0]

            def fetch_upto(n):
                while fetched[0] < min(n, len(fetch_seq)):
                    p_, qc_ = fetch_seq[fetched[0]]
                    kt = kpool.tile([128, 512], BF16, name="kTc", tag="kTc")
                    nc.gpsimd.dma_start(out=kt, in_=ktd[(p_, qc_)])
                    kch[(p_, qc_)] = kt
                    qt = qpool.tile([128, 512], BF16, name="qTc", tag="qTc")
                    nc.gpsimd.dma_start(out=qt, in_=qtd[(p_, qc_)])
                    qch[(p_, qc_)] = qt
                    fetched[0] += 1

            # ---------------- phase 1: projections ----------------
            with (
                tc.tile_pool(name="ph1", bufs=1) as ph1,
                tc.tile_pool(name="bounce", bufs=3) as bpool,
                tc.tile_pool(name="psp", bufs=6, space="PSUM") as psp,
            ):
                wv_sb = ph1.tile([128, CT, 512], F32R)
                wq_sb = ph1.tile([128, CT, NP, 128], F32R)
                wk_sb = ph1.tile([128, CT, NP, 128], F32R)
                xT = [ph1.tile([128, T], F32R, name=f"xT{cc}") for cc in range(CT)]
                cp_flip = [0]

                def copy_alt(out, in_):
                    eng = cp_flip[0] % 2
                    cp_flip[0] += 1
                    if eng == 0:
                        nc.vector.tensor_copy(out=out, in_=in_)
                    else:
                        nc.scalar.copy(out=out, in_=in_)

                def load_xt_chunk(tc_):
                    for cc in range(CT):
                        nc.sync.dma_start(
                            out=xT[cc][:, ts(tc_, 512)],
                            in_=xat[ts(cc, 128), ts(tc_, 512)].bitcast(F32R),
                        )

                def load_weights():
                    nc.gpsimd.dma_start(out=tr, in_=tri)
                    nc.gpsimd.dma_start(
                        out=wv_sb,
                        in_=wv.rearrange("(ct P) f -> P ct f", P=128).bitcast(F32R),
                    )
                    nc.gpsimd.dma_start(
                        out=wq_sb,
                        in_=wq.rearrange("(ct P) (np f) -> P ct np f", P=128, np=NP).bitcast(F32R),
                    )
                    nc.gpsimd.dma_start(
                        out=wk_sb,
                        in_=wk.rearrange("(ct P) (np f) -> P ct np f", P=128, np=NP).bitcast(F32R),
                    )
                    nc.vector.tensor_copy(out=tr16, in_=tr)

                def load_wo():
                    wo_f = ph1.tile([128, NP, 1024], F32)
                    nc.gpsimd.dma_start(
                        out=wo_f, in_=wo.rearrange("(np P) f -> P np f", P=128)
                    )
                    nc.vector.tensor_copy(out=wo_sb, in_=wo_f)

                def vproj_tt(tt):
                    ps = psp.tile([128, 512], F32)
                    for cc in range(CT):
                        mm(
                            ps,
                            lhsT=xT[cc][:, ts(tt, 128)],
                            rhs=wv_sb[:, cc, :],
                            start=(cc == 0),
                            stop=(cc == CT - 1),
                        )
                    nc.vector.tensor_copy(
                        out=vS[:, tt, :, 0:64],
                        in_=ps.rearrange("p (h d) -> p h d", h=8),
                    )

                def qkproj_tc(tc_):
                    for p in range(NP):
                        for w_sb, dst in ((wq_sb, qtd), (wk_sb, ktd)):
                            ps = psp.tile([128, 512], F32)
                            for cc in range(CT):
                                mm(
                                    ps,
                                    lhsT=w_sb[:, cc, p, :],
                                    rhs=xT[cc][:, ts(tc_, 512)],
                                    start=(cc == 0),
                                    stop=(cc == CT - 1),
                                )
                            bo = bpool.tile([128, 512], BF16)
                            copy_alt(bo, ps)
                            nc.scalar.dma_start(out=dst[(p, tc_)], in_=bo)

                load_xt_chunk(0)
                load_weights()
                load_wo()
                for tc_ in range(4):
                    if tc_ < 3:
                        load_xt_chunk(tc_ + 1)
                    for tt in range(4 * tc_, 4 * tc_ + 4):
                        vproj_tt(tt)
                    qkproj_tc(tc_)
                    fetch_upto(tc_ + 1)

            # ---------------- phase 2: attention ----------------
            with (
                tc.tile_pool(name="ptp", bufs=12) as ptpool,
                tc.tile_pool(name="tmp", bufs=6) as tmppool,
                tc.tile_pool(name="rsm", bufs=6) as rpool,
                tc.tile_pool(name="rbcp", bufs=4) as rbcpool,
                tc.tile_pool(name="psS", bufs=3, space="PSUM") as psS,
                tc.tile_pool(name="psAv", bufs=2, space="PSUM") as psAv,
            ):
                avs = {}
                pts = {}
                eng_load = [0.0, 0.0]  # ns accumulated: [ACT, DVE]
                trb16 = tr16[:, None, :].to_broadcast([128, 2, 128])

                def s_exp(p, qc, j):
                    off = max(0, 128 * j - 512 * qc)
                    sg = psS.tile([128, 2, 512], F32)
                    kc = kch[(p, j // 4)]
                    qc_t = qch[(p, qc)]
                    jo = 128 * (j % 4)
                    for m in range(2):
                        mm(
                            sg[:, m, off:],
                            lhsT=kc[64 * m : 64 * m + 64, jo : jo + 128],
                            rhs=qc_t[64 * m : 64 * m + 64, off:],
                            start=True,
                            stop=True,
                        )
                    ptile = ptpool.tile([128, 2, 512], BF16)
                    masked = j >= 4 * qc
                    cols = 2 * (512 - off)
                    cost_a = cols * 1.30 + 150.0
                    cost_d = cols * 1.90 + 300.0
                    if masked:
                        cost_a += 420.0  # DVE mask-mul it will require
                    use_act = eng_load[0] + cost_a <= eng_load[1] + cost_d
                    if use_act:
                        eng_load[0] += cols * 1.30 + 150.0
                        nc.scalar.activation(
                            out=ptile[:, :, off:], in_=sg[:, :, off:],
                            func=EXP, scale=LN2,
                        )
                        if masked:
                            eng_load[1] += 420.0
                            nc.vector.tensor_mul(
                                ptile[:, :, off : off + 128],
                                ptile[:, :, off : off + 128],
                                trb16,
                            )
                    else:
                        eng_load[1] += cost_d
                        tmp = tmppool.tile([128, 2, 512], F32)
                        nc.vector._custom_dve(
                            EXP_P1, out=tmp[:, :, off:], in0=sg[:, :, off:],
                            imm2=LN2 / EXP_N,
                        )
                        if masked:
                            nc.vector._custom_dve(
                                EXP_SQ7M,
                                out=ptile[:, :, off : off + 128],
                                in0=tmp[:, :, off : off + 128],
                                in1=trb16,
                            )
                            if off + 128 < 512:
                                nc.vector._custom_dve(
                                    EXP_SQ7,
                                    out=ptile[:, :, off + 128 :],
                                    in0=tmp[:, :, off + 128 :],
                                )
                        else:
                            nc.vector._custom_dve(
                                EXP_SQ7, out=ptile[:, :, off:], in0=tmp[:, :, off:]
                            )
                    pts[(p, qc, j)] = (ptile, off)

                def av_mm(p, qc, j, nj):
                    ptile, off = pts.pop((p, qc, j))
                    av = avs[(p, qc)]
                    for m in range(2):
                        mm(
                            av[m][:65, off:],
                            lhsT=vS[:, j, 2 * p + m, :],
                            rhs=ptile[:, m, off:],
                            start=(j == 0),
                            stop=(j == nj - 1),
                        )

                def normalize(p, qc):
                    av = avs.pop((p, qc))
                    rinvs = []
                    for m in range(2):
                        rsb = rpool.tile([1, 512], F32, name="rsb", tag="rsb")
                        nc.vector.tensor_copy(out=rsb, in_=av[m][64:65, :])
                        rinv = rpool.tile([1, 512], F32, name="rinv", tag="rinv")
                        nc.vector.reciprocal_approx_fast(out=rinv, in_=rsb)
                        rinvs.append(rinv)
                    for m in range(2):
                        rb = rbcpool.tile([128, 512], F32, name="rb", tag="rb")
                        nc.gpsimd.partition_broadcast(rb, rinvs[m])
                        dst = OT[64 * m : 64 * m + 64, p, ts(qc, 512)]
                        if m == 0:
                            # all operands base-0 aligned: fused copy+normalize
                            nc.vector.tensor_mul(dst, av[m][0:64, :], rb[0:64, :])
                            eng_load[1] += 800.0
                        else:
                            nc.vector.tensor_copy(out=dst, in_=av[m][0:64, :])
                            nc.vector.tensor_mul(dst, dst, rb[64:128, :])
                            eng_load[1] += 1600.0

                groups = []
                for p in range(NP):
                    for qc in range(NQC):
                        nj = 4 * qc + 4
                        js = list(range(nj))
                        sub = [js[i : i + 3] for i in range(0, nj, 3)]
                        for gi, jg in enumerate(sub):
                            groups.append((p, qc, nj, jg, gi == 0, gi == len(sub) - 1))

                def av_group(gi):
                    p, qc, nj, jg, first, last = groups[gi]
                    if first:
                        avs[(p, qc)] = [
                            psAv.tile([128, 512], F32, name="av", tag="av")
                            for _ in range(2)
                        ]
                    for j in jg:
                        av_mm(p, qc, j, nj)
                    if last:
                        normalize(p, qc)

                LAG = 2
                seq_pos = {(pq): i for i, pq in enumerate(fetch_seq)}
                for i in range(len(groups) + LAG):
                    if i < len(groups):
                        p, qc, nj, jg, first, last = groups[i]
                        if first:
                            fetch_upto(seq_pos[(p, qc)] + 3)
                        for j in jg:
                            s_exp(p, qc, j)
                    if i >= LAG and (i - LAG) % 2 == 1:
                        av_group(i - LAG - 1)
                        av_group(i - LAG)
                if len(groups) % 2 == 1:
                    av_group(len(groups) - 1)

            # ---------------- phase 3: output projection ----------------
            with (
                tc.tile_pool(name="obnc", bufs=4) as opool,
                tc.tile_pool(name="psO", bufs=8, space="PSUM") as psO,
            ):
                ob_flip = [0]
                for tc_ in range(4):
                    for ft in range(8):
                        pso = psO.tile([128, 512], F32, name="pso", tag="pso")
                        for p in range(NP):
                            mm(
                                pso,
                                lhsT=wo_sb[:, p, ts(ft, 128)],
                                rhs=OT[:, p, ts(tc_, 512)],
                                start=(p == 0),
                                stop=(p == NP - 1),
                            )
                        ob = opool.tile([128, 512], F32)
                        if ob_flip[0] % 2 == 0:
                            nc.vector.tensor_copy(out=ob, in_=pso)
                        else:
                            nc.scalar.copy(out=ob, in_=pso)
                        ob_flip[0] += 1
                        nc.sync.dma_start(out=ot[ts(ft, 128), ts(tc_, 512)], in_=ob)

    nc.compile()
    return nc


def kernel(x, W_qkv, b_qkv, W_out, b_out):
    global _last_in_maps
    x = np.asarray(x, dtype=np.float32)
    W_qkv = np.asarray(W_qkv, dtype=np.float32)
    b_qkv = np.asarray(b_qkv, dtype=np.float32)
    W_out = np.asarray(W_out, dtype=np.float32)
    b_out = np.asarray(b_out, dtype=np.float32)
    B = x.shape[0]

    aug = bool(np.any(b_qkv))
    CT = 9 if aug else 8
    if CT not in _CACHE:
        _CACHE[CT] = _build(CT)
    nc = _CACHE[CT]

    # triangle keep-mask for the diagonal 128 block: [p, c] = 1 if c >= p
    tri = (np.arange(128)[None, :] >= np.arange(128)[:, None]).astype(np.float32)

    in_maps = []
    for core in range(8):
        b, g = core // 2, core % 2
        xa = x[b]
        if aug:
            pad = np.zeros((T, 128), np.float32)
            pad[:, 0] = 1.0
            xa = np.concatenate([xa, pad], axis=1)

        def wslice(col0, qscale=1.0):
            w = W_qkv[:, col0 + 512 * g : col0 + 512 * g + 512]
            if aug:
                extra = np.zeros((128, 512), np.float32)
                extra[0] = b_qkv[col0 + 512 * g : col0 + 512 * g + 512]
                w = np.concatenate([w, extra], axis=0)
            if qscale != 1.0:
                w = w * np.float32(qscale)
            return np.ascontiguousarray(w)

        in_maps.append(
            {
                "xat": np.ascontiguousarray(xa.T),
                "wq": wslice(0, qscale=SCALE * LOG2E),
                "wk": wslice(1024),
                "wv": wslice(2048),
                "wo": np.ascontiguousarray(W_out[512 * g : 512 * g + 512, :]),
                "tri": tri,
            }
        )

    _last_in_maps = in_maps
    res = bass_utils.run_bass_kernel_spmd(nc, in_maps, list(range(8))).results
    out = np.empty((B, T, 1024), np.float32)
    for b in range(B):
        acc = res[2 * b]["ot"] + res[2 * b + 1]["ot"]
        out[b] = acc.T + b_out[None, :]
    return out


# revision 19
# speedup vs baseline: 1.6249x; 1.0110x over previous
"""Causal self-attention (B=4, T=2048, D=1024, H=16) on 8 TRN2 NeuronCores.

Sharding: core i = (batch b = i//2, head-group g = i%2). Data parallel on B,
tensor parallel on heads (8 heads per group): qkv_proj columns and out_proj
rows split per head group. Each core computes a partial [D, T] output^T for
its batch; host sums the two group partials per batch, transposes, adds bias.

v3 design (baseline 396-405us):
  - x is transposed on the HOST (xat = x[b].T): no PE transposes at all.
    Phase 1 streams xT chunks in 512-col slices and runs V/Q/K projections.
  - softmax runs in the log2 domain: W_q is host-folded with 0.125*log2(e),
    so S = log2-weights. ACT exp uses scale=ln2; the custom DVE limit-exp
    computes (1 + y*ln2/8192)^8192 in two passes (8+7 squarings), causal
    mask fused into the DVE pass-2 where assigned.
  - exp strips are split ACT/DVE by greedy load balance (ACT ~1.3ns/col,
    DVE 2-pass ~1.9ns/col measured). gpsimd does ONLY partition_broadcast
    (+DMA issue): mixing op types thrashes its ucode library (~7us/swap).
  - Q/K bounce, S matmul operands, P, V, O~, Wo are bf16. bf16 K=64 S
    matmuls row-pack the two heads concurrently (f32r would need 2x rows).
  - per-(pair,qc) Q/K chunk prefetch 2 steps ahead (starting in phase 1)
    keeps the PE continuously fed (no HAM re-throttle).
"""

import numpy as np

import concourse.bacc as bacc
import concourse.tile as tile
import concourse.mybir as mybir
from concourse import bass_utils
from concourse.bass import ts

F32 = mybir.dt.float32
F32R = mybir.dt.float32r
BF16 = mybir.dt.bfloat16
EXP = mybir.ActivationFunctionType.Exp

T = 2048
TT = 16          # t tiles of 128
NP = 4           # head pairs per core
NQC = 4          # q chunks of 512
SCALE = 0.125    # 1/sqrt(64)
LOG2E = float(np.log2(np.e))
LN2 = float(np.log(2.0))
EXP_N = 8192     # limit-exp exponent: (1 + u/EXP_N)^EXP_N ~= exp(u)

_CACHE = {}
_last_in_maps = None

# ---------------------------------------------------------------------------
# custom DVE exp:  pass1  t = (1 + y*imm)^64        (mul, add, 6 squarings)
#                  pass2  p = t^128                 (7 squarings)
#                  pass2m p = t^128 * mask          (7 squarings + mul)
# with imm = ln2/8192 this computes ~2^y for log2-domain scores y.
_EXP_OPS = None


def _ensure_exp_ops():
    global _EXP_OPS
    if _EXP_OPS is not None:
        return _EXP_OPS
    from concourse import dve_ops as dops
    from concourse.dve_spec import Spec, Src0, Src1, One, C2, lower
    from concourse.dve_spec import _has_src1 as has_src1
    from concourse.dve_uop import DveOpSpec
    from concourse.dve_table_gen import dve_ver_for

    ver = dve_ver_for("TRN2")
    existing = {o.name: o for o in dops.OPS}

    def mk(name, spec):
        if name in existing:
            return existing[name]
        row = max(dops._SUB_OPCODE_FOR_NAME.values()) + 1
        assert row < 0x20, "custom-DVE opcode rows exhausted"
        dops._SUB_OPCODE_FOR_NAME[name] = row
        s = DveOpSpec(
            name=name, opcode=row, uops=lower(spec, ver=ver),
            rd1_en=has_src1(spec),
        )
        op = dops.DveOp(name, spec, subdim=False, uops_sha={ver: s.sha(ver)})
        dops.OPS.append(op)
        dops.CUSTOM_DVE_SPECS[name] = spec
        return op

    p = One + Src0 * C2
    for _ in range(6):
        p = p * p
    q = Src0
    for _ in range(7):
        q = q * q
    qm = Src0
    for _ in range(7):
        qm = qm * qm
    _EXP_OPS = (
        mk("ANT_EXP_P1", Spec(body=p)),
        mk("ANT_EXP_SQ7", Spec(body=q)),
        mk("ANT_EXP_SQ7M", Spec(body=qm * Src1)),
    )
    return _EXP_OPS


def _build(CT):
    """CT = number of 128-row c-tiles in the (possibly bias-augmented) x/W."""
    EXP_P1, EXP_SQ7, EXP_SQ7M = _ensure_exp_ops()
    nc = bacc.Bacc("TRN2", target_bir_lowering=False, debug=False)
    C = CT * 128

    xat = nc.dram_tensor("xat", [C, T], F32, kind="ExternalInput").ap()
    wq = nc.dram_tensor("wq", [C, 512], F32, kind="ExternalInput").ap()
    wk = nc.dram_tensor("wk", [C, 512], F32, kind="ExternalInput").ap()
    wv = nc.dram_tensor("wv", [C, 512], F32, kind="ExternalInput").ap()
    wo = nc.dram_tensor("wo", [512, 1024], F32, kind="ExternalInput").ap()
    tri = nc.dram_tensor("tri", [128, 128], F32, kind="ExternalInput").ap()
    ot = nc.dram_tensor("ot", [1024, T], F32, kind="ExternalOutput").ap()

    mm = nc.tensor.matmul

    with tile.TileContext(nc) as tc:
        with (
            tc.tile_pool(name="persist", bufs=1) as persist,
            tc.tile_pool(name="dram", bufs=1, space="DRAM") as dpool,
            tc.tile_pool(name="kch", bufs=6) as kpool,
            tc.tile_pool(name="qch", bufs=4) as qpool,
        ):
            vS = persist.tile([128, TT, 8, 65], BF16)     # [k128, ktile, head, d+1]
            OT = persist.tile([128, NP, T], BF16)         # [d128(2 heads), pair, t]
            tr = persist.tile([128, 128], F32)            # keep-mask f32
            tr16 = persist.tile([128, 128], BF16)         # keep-mask bf16
            wo_sb = persist.tile([128, NP, 1024], BF16)
            nc.vector.memset(vS[:, :, :, 64:65], 1.0)

            # Q^T / K^T bounce chunks (bf16), one DRAM tile per (pair, t-chunk)
            qtd = {}
            ktd = {}
            for _p in range(NP):
                for _tc in range(4):
                    qtd[(_p, _tc)] = dpool.tile([128, 512], BF16, name=f"qtd{_p}{_tc}")
                    ktd[(_p, _tc)] = dpool.tile([128, 512], BF16, name=f"ktd{_p}{_tc}")

            qch = {}
            kch = {}
            fetch_seq = [(p, qc) for p in range(NP) for qc in range(NQC)]
            fetched = [0]

            def fetch_upto(n):
                while fetched[0] < min(n, len(fetch_seq)):
                    p_, qc_ = fetch_seq[fetched[0]]
                    kt = kpool.tile([128, 512], BF16, name="kTc", tag="kTc")
                    nc.sync.dma_start(out=kt, in_=ktd[(p_, qc_)])
                    kch[(p_, qc_)] = kt
                    qt = qpool.tile([128, 512], BF16, name="qTc", tag="qTc")
                    nc.sync.dma_start(out=qt, in_=qtd[(p_, qc_)])
                    qch[(p_, qc_)] = qt
                    fetched[0] += 1

            # ---------------- phase 1: projections ----------------
            with (
                tc.tile_pool(name="ph1", bufs=1) as ph1,
                tc.tile_pool(name="bounce", bufs=3) as bpool,
                tc.tile_pool(name="psp", bufs=6, space="PSUM") as psp,
            ):
                wv_sb = ph1.tile([128, CT, 512], F32R)
                wq_sb = ph1.tile([128, CT, NP, 128], F32R)
                wk_sb = ph1.tile([128, CT, NP, 128], F32R)
                # one tile per (cc, tc) chunk: a shared per-cc tile would add
                # false WAR deps (chunk tc+1's DMA waits on tc's readers)
                xT = {
                    (cc, tc_): ph1.tile([128, 512], F32R, name=f"xT{cc}_{tc_}")
                    for cc in range(CT)
                    for tc_ in range(4)
                }
                cp_flip = [0]

                def copy_alt(out, in_):
                    eng = cp_flip[0] % 2
                    cp_flip[0] += 1
                    if eng == 0:
                        nc.vector.tensor_copy(out=out, in_=in_)
                    else:
                        nc.scalar.copy(out=out, in_=in_)

                def load_xt_chunk(tc_):
                    for cc in range(CT):
                        nc.sync.dma_start(
                            out=xT[(cc, tc_)],
                            in_=xat[ts(cc, 128), ts(tc_, 512)].bitcast(F32R),
                        )

                def load_weights():
                    nc.gpsimd.dma_start(out=tr, in_=tri)
                    nc.gpsimd.dma_start(
                        out=wv_sb,
                        in_=wv.rearrange("(ct P) f -> P ct f", P=128).bitcast(F32R),
                    )
                    nc.gpsimd.dma_start(
                        out=wq_sb,
                        in_=wq.rearrange("(ct P) (np f) -> P ct np f", P=128, np=NP).bitcast(F32R),
                    )
                    nc.gpsimd.dma_start(
                        out=wk_sb,
                        in_=wk.rearrange("(ct P) (np f) -> P ct np f", P=128, np=NP).bitcast(F32R),
                    )
                    nc.vector.tensor_copy(out=tr16, in_=tr)

                def load_wo():
                    wo_f = ph1.tile([128, NP, 1024], F32)
                    nc.gpsimd.dma_start(
                        out=wo_f, in_=wo.rearrange("(np P) f -> P np f", P=128)
                    )
                    nc.vector.tensor_copy(out=wo_sb, in_=wo_f)

                def vproj_tt(tt):
                    ps = psp.tile([128, 512], F32)
                    for cc in range(CT):
                        mm(
                            ps,
                            lhsT=xT[(cc, tt // 4)][:, ts(tt % 4, 128)],
                            rhs=wv_sb[:, cc, :],
                            start=(cc == 0),
                            stop=(cc == CT - 1),
                        )
                    nc.vector.tensor_copy(
                        out=vS[:, tt, :, 0:64],
                        in_=ps.rearrange("p (h d) -> p h d", h=8),
                    )

                def qkproj_tc(tc_):
                    for p in range(NP):
                        for w_sb, dst in ((wq_sb, qtd), (wk_sb, ktd)):
                            ps = psp.tile([128, 512], F32)
                            for cc in range(CT):
                                mm(
                                    ps,
                                    lhsT=w_sb[:, cc, p, :],
                                    rhs=xT[(cc, tc_)],
                                    start=(cc == 0),
                                    stop=(cc == CT - 1),
                                )
                            bo = bpool.tile([128, 512], BF16)
                            copy_alt(bo, ps)
                            nc.scalar.dma_start(out=dst[(p, tc_)], in_=bo)

                load_xt_chunk(0)
                load_weights()
                load_wo()
                for tc_ in range(4):
                    if tc_ < 3:
                        load_xt_chunk(tc_ + 1)
                    for tt in range(4 * tc_, 4 * tc_ + 4):
                        vproj_tt(tt)
                    qkproj_tc(tc_)
                    fetch_upto(tc_ + 1)

            # ---------------- phase 2: attention ----------------
            with (
                tc.tile_pool(name="ptp", bufs=12) as ptpool,
                tc.tile_pool(name="tmp", bufs=6) as tmppool,
                tc.tile_pool(name="rsm", bufs=6) as rpool,
                tc.tile_pool(name="rbcp", bufs=4) as rbcpool,
                tc.tile_pool(name="psS", bufs=3, space="PSUM") as psS,
                tc.tile_pool(name="psAv", bufs=2, space="PSUM") as psAv,
            ):
                avs = {}
                pts = {}
                eng_load = [0.0, 0.0]  # ns accumulated: [ACT, DVE]
                trb16 = tr16[:, None, :].to_broadcast([128, 2, 128])

                def s_exp(p, qc, j):
                    off = max(0, 128 * j - 512 * qc)
                    sg = psS.tile([128, 2, 512], F32)
                    kc = kch[(p, j // 4)]
                    qc_t = qch[(p, qc)]
                    jo = 128 * (j % 4)
                    for m in range(2):
                        mm(
                            sg[:, m, off:],
                            lhsT=kc[64 * m : 64 * m + 64, jo : jo + 128],
                            rhs=qc_t[64 * m : 64 * m + 64, off:],
                            start=True,
                            stop=True,
                        )
                    ptile = ptpool.tile([128, 2, 512], BF16)
                    masked = j >= 4 * qc
                    cols = 2 * (512 - off)
                    cost_a = cols * 1.08 + 150.0
                    cost_d = cols * 1.56 + 300.0
                    if masked:
                        cost_a += 420.0  # DVE mask-mul it will require
                    use_act = eng_load[0] + cost_a <= eng_load[1] + cost_d
                    if use_act:
                        eng_load[0] += cols * 1.08 + 150.0
                        nc.scalar.activation(
                            out=ptile[:, :, off:], in_=sg[:, :, off:],
                            func=EXP, scale=LN2,
                        )
                        if masked:
                            eng_load[1] += 420.0
                            nc.vector.tensor_mul(
                                ptile[:, :, off : off + 128],
                                ptile[:, :, off : off + 128],
                                trb16,
                            )
                    else:
                        eng_load[1] += cost_d
                        tmp = tmppool.tile([128, 2, 512], F32)
                        nc.vector._custom_dve(
                            EXP_P1, out=tmp[:, :, off:], in0=sg[:, :, off:],
                            imm2=LN2 / EXP_N,
                        )
                        if masked:
                            nc.vector._custom_dve(
                                EXP_SQ7M,
                                out=ptile[:, :, off : off + 128],
                                in0=tmp[:, :, off : off + 128],
                                in1=trb16,
                            )
                            if off + 128 < 512:
                                nc.vector._custom_dve(
                                    EXP_SQ7,
                                    out=ptile[:, :, off + 128 :],
                                    in0=tmp[:, :, off + 128 :],
                                )
                        else:
                            nc.vector._custom_dve(
                                EXP_SQ7, out=ptile[:, :, off:], in0=tmp[:, :, off:]
                            )
                    pts[(p, qc, j)] = (ptile, off)

                def av_mm(p, qc, j, nj):
                    ptile, off = pts.pop((p, qc, j))
                    av = avs[(p, qc)]
                    for m in range(2):
                        mm(
                            av[m][:65, off:],
                            lhsT=vS[:, j, 2 * p + m, :],
                            rhs=ptile[:, m, off:],
                            start=(j == 0),
                            stop=(j == nj - 1),
                        )

                def normalize(p, qc):
                    av = avs.pop((p, qc))
                    rinvs = []
                    for m in range(2):
                        rsb = rpool.tile([1, 512], F32, name="rsb", tag="rsb")
                        nc.vector.tensor_copy(out=rsb, in_=av[m][64:65, :])
                        rinv = rpool.tile([1, 512], F32, name="rinv", tag="rinv")
                        nc.vector.reciprocal_approx_fast(out=rinv, in_=rsb)
                        rinvs.append(rinv)
                    for m in range(2):
                        rb = rbcpool.tile([128, 512], F32, name="rb", tag="rb")
                        nc.gpsimd.partition_broadcast(rb, rinvs[m])
                        dst = OT[64 * m : 64 * m + 64, p, ts(qc, 512)]
                        if m == 0:
                            # all operands base-0 aligned: fused copy+normalize
                            nc.vector.tensor_mul(dst, av[m][0:64, :], rb[0:64, :])
                            eng_load[1] += 800.0
                        else:
                            nc.vector.tensor_copy(out=dst, in_=av[m][0:64, :])
                            nc.vector.tensor_mul(dst, dst, rb[64:128, :])
                            eng_load[1] += 1600.0

                groups = []
                for p in range(NP):
                    for qc in range(NQC):
                        nj = 4 * qc + 4
                        js = list(range(nj))
                        sub = [js[i : i + 3] for i in range(0, nj, 3)]
                        for gi, jg in enumerate(sub):
                            groups.append((p, qc, nj, jg, gi == 0, gi == len(sub) - 1))

                def av_group(gi):
                    p, qc, nj, jg, first, last = groups[gi]
                    if first:
                        avs[(p, qc)] = [
                            psAv.tile([128, 512], F32, name="av", tag="av")
                            for _ in range(2)
                        ]
                    for j in jg:
                        av_mm(p, qc, j, nj)
                    if last:
                        normalize(p, qc)

                LAG = 2
                seq_pos = {(pq): i for i, pq in enumerate(fetch_seq)}
                for i in range(len(groups) + LAG):
                    if i < len(groups):
                        p, qc, nj, jg, first, last = groups[i]
                        if first:
                            fetch_upto(seq_pos[(p, qc)] + 3)
                        for j in jg:
                            s_exp(p, qc, j)
                    if i >= LAG and (i - LAG) % 2 == 1:
                        av_group(i - LAG - 1)
                        av_group(i - LAG)
                if len(groups) % 2 == 1:
                    av_group(len(groups) - 1)

            # ---------------- phase 3: output projection ----------------
            with (
                tc.tile_pool(name="obnc", bufs=4) as opool,
                tc.tile_pool(name="psO", bufs=8, space="PSUM") as psO,
            ):
                ob_flip = [0]
                for tc_ in range(4):
                    for ft in range(8):
                        pso = psO.tile([128, 512], F32, name="pso", tag="pso")
                        for p in range(NP):
                            mm(
                                pso,
                                lhsT=wo_sb[:, p, ts(ft, 128)],
                                rhs=OT[:, p, ts(tc_, 512)],
                                start=(p == 0),
                                stop=(p == NP - 1),
                            )
                        ob = opool.tile([128, 512], F32)
                        if ob_flip[0] % 2 == 0:
                            nc.vector.tensor_copy(out=ob, in_=pso)
                        else:
                            nc.scalar.copy(out=ob, in_=pso)
                        ob_flip[0] += 1
                        nc.sync.dma_start(out=ot[ts(ft, 128), ts(tc_, 512)], in_=ob)

    nc.compile()
    return nc


def kernel(x, W_qkv, b_qkv, W_out, b_out):
    global _last_in_maps
    x = np.asarray(x, dtype=np.float32)
    W_qkv = np.asarray(W_qkv, dtype=np.float32)
    b_qkv = np.asarray(b_qkv, dtype=np.float32)
    W_out = np.asarray(W_out, dtype=np.float32)
    b_out = np.asarray(b_out, dtype=np.float32)
    B = x.shape[0]

    aug = bool(np.any(b_qkv))
    CT = 9 if aug else 8
    if CT not in _CACHE:
        _CACHE[CT] = _build(CT)
    nc = _CACHE[CT]

    # triangle keep-mask for the diagonal 128 block: [p, c] = 1 if c >= p
    tri = (np.arange(128)[None, :] >= np.arange(128)[:, None]).astype(np.float32)

    in_maps = []
    for core in range(8):
        b, g = core // 2, core % 2
        xa = x[b]
        if aug:
            pad = np.zeros((T, 128), np.float32)
            pad[:, 0] = 1.0
            xa = np.concatenate([xa, pad], axis=1)

        def wslice(col0, qscale=1.0):
            w = W_qkv[:, col0 + 512 * g : col0 + 512 * g + 512]
            if aug:
                extra = np.zeros((128, 512), np.float32)
                extra[0] = b_qkv[col0 + 512 * g : col0 + 512 * g + 512]
                w = np.concatenate([w, extra], axis=0)
            if qscale != 1.0:
                w = w * np.float32(qscale)
            return np.ascontiguousarray(w)

        in_maps.append(
            {
                "xat": np.ascontiguousarray(xa.T),
                "wq": wslice(0, qscale=SCALE * LOG2E),
                "wk": wslice(1024),
                "wv": wslice(2048),
                "wo": np.ascontiguousarray(W_out[512 * g : 512 * g + 512, :]),
                "tri": tri,
            }
        )

    _last_in_maps = in_maps
    res = bass_utils.run_bass_kernel_spmd(nc, in_maps, list(range(8))).results
    out = np.empty((B, T, 1024), np.float32)
    for b in range(B):
        acc = res[2 * b]["ot"] + res[2 * b + 1]["ot"]
        out[b] = acc.T + b_out[None, :]
    return out


# revision 21
# speedup vs baseline: 1.8229x; 1.1219x over previous
"""Causal self-attention (B=4, T=2048, D=1024, H=16) on 8 TRN2 NeuronCores.

Sharding: core i = (batch b = i//2, head-group g = i%2). Data parallel on B,
tensor parallel on heads (8 heads per group): qkv_proj columns and out_proj
rows split per head group. Each core computes a partial [D, T] output^T for
its batch; host sums the two group partials per batch, transposes, adds bias.

v5 design (baseline 396-405us):
  - ALL device inputs are bf16, pre-transposed/pre-arranged into their SBUF
    layouts on the host: x^T [C,T], weights in [P, ...] partition layout.
    Phase-1 DMA drops to ~9MB contiguous; no PE transposes; phase-1
    projections run all-bf16 (FWL weight loads).
  - softmax in the log2 domain: W_q host-folded with 0.125*log2(e). ACT exp
    uses scale=ln2; custom DVE limit-exp (1 + y*ln2/8192)^8192 in two
    passes, causal mask fused into DVE pass-2 where assigned; exp strips
    split ACT/DVE by measured-rate greedy (1.08 vs 1.75 ns/col).
  - gpsimd does ONLY partition_broadcast: any other op type thrashes its
    ucode library (~7us per swap). Chunk fetches ride the sync queue.
  - bf16 K=64 S matmuls row-pack both heads concurrently (f32r cannot).
  - psAv has 4 banks so the normalize chain (copy/recip/broadcast/mul) is
    off the PE critical path at (pair,qc) boundaries; psS double-buffered.
  - per-(pair,qc) Q/K chunk prefetch 2 steps ahead starting in phase 1.
"""

import numpy as np
import ml_dtypes

import concourse.bacc as bacc
import concourse.tile as tile
import concourse.mybir as mybir
from concourse import bass_utils
from concourse.bass import ts

F32 = mybir.dt.float32
BF16 = mybir.dt.bfloat16
EXP = mybir.ActivationFunctionType.Exp
BF = ml_dtypes.bfloat16

T = 2048
TT = 16          # t tiles of 128
NP = 4           # head pairs per core
NQC = 4          # q chunks of 512
SCALE = 0.125    # 1/sqrt(64)
LOG2E = float(np.log2(np.e))
LN2 = float(np.log(2.0))
EXP_N = 8192     # limit-exp exponent: (1 + u/EXP_N)^EXP_N ~= exp(u)

_CACHE = {}
_last_in_maps = None

_EXP_OPS = None


def _ensure_exp_ops():
    global _EXP_OPS
    if _EXP_OPS is not None:
        return _EXP_OPS
    from concourse import dve_ops as dops
    from concourse.dve_spec import Spec, Src0, Src1, One, C2, lower
    from concourse.dve_spec import _has_src1 as has_src1
    from concourse.dve_uop import DveOpSpec
    from concourse.dve_table_gen import dve_ver_for

    ver = dve_ver_for("TRN2")
    existing = {o.name: o for o in dops.OPS}

    def mk(name, spec):
        if name in existing:
            return existing[name]
        row = max(dops._SUB_OPCODE_FOR_NAME.values()) + 1
        assert row < 0x20, "custom-DVE opcode rows exhausted"
        dops._SUB_OPCODE_FOR_NAME[name] = row
        s = DveOpSpec(
            name=name, opcode=row, uops=lower(spec, ver=ver),
            rd1_en=has_src1(spec),
        )
        op = dops.DveOp(name, spec, subdim=False, uops_sha={ver: s.sha(ver)})
        dops.OPS.append(op)
        dops.CUSTOM_DVE_SPECS[name] = spec
        return op

    p = One + Src0 * C2
    for _ in range(6):
        p = p * p
    q = Src0
    for _ in range(7):
        q = q * q
    qm = Src0
    for _ in range(7):
        qm = qm * qm
    _EXP_OPS = (
        mk("ANT_EXP_P1", Spec(body=p)),
        mk("ANT_EXP_SQ7", Spec(body=q)),
        mk("ANT_EXP_SQ7M", Spec(body=qm * Src1)),
    )
    return _EXP_OPS


def _build(CT):
    """CT = number of 128-row c-tiles in the (possibly bias-augmented) x/W."""
    EXP_P1, EXP_SQ7, EXP_SQ7M = _ensure_exp_ops()
    nc = bacc.Bacc("TRN2", target_bir_lowering=False, debug=False)
    C = CT * 128

    xat = nc.dram_tensor("xat", [C, T], BF16, kind="ExternalInput").ap()
    wqr = nc.dram_tensor("wqr", [128, CT, NP, 128], BF16, kind="ExternalInput").ap()
    wkr = nc.dram_tensor("wkr", [128, CT, NP, 128], BF16, kind="ExternalInput").ap()
    wvr = nc.dram_tensor("wvr", [128, CT, 512], BF16, kind="ExternalInput").ap()
    wor = nc.dram_tensor("wor", [128, NP, 1024], BF16, kind="ExternalInput").ap()
    tri = nc.dram_tensor("tri", [128, 128], BF16, kind="ExternalInput").ap()
    ot = nc.dram_tensor("ot", [1024, T], F32, kind="ExternalOutput").ap()

    mm = nc.tensor.matmul

    with tile.TileContext(nc) as tc:
        with (
            tc.tile_pool(name="persist", bufs=1) as persist,
            tc.tile_pool(name="dram", bufs=1, space="DRAM") as dpool,
            tc.tile_pool(name="kch", bufs=6) as kpool,
            tc.tile_pool(name="qch", bufs=4) as qpool,
        ):
            vS = persist.tile([128, TT, 8, 65], BF16)     # [k128, ktile, head, d+1]
            OT = persist.tile([128, NP, T], BF16)         # [d128(2 heads), pair, t]
            tr16 = persist.tile([128, 128], BF16)         # keep-mask
            wo_sb = persist.tile([128, NP, 1024], BF16)
            nc.vector.memset(vS[:, :, :, 64:65], 1.0)

            qtd = {}
            ktd = {}
            for _p in range(NP):
                for _tc in range(4):
                    qtd[(_p, _tc)] = dpool.tile([128, 512], BF16, name=f"qtd{_p}{_tc}")
                    ktd[(_p, _tc)] = dpool.tile([128, 512], BF16, name=f"ktd{_p}{_tc}")

            qch = {}
            kch = {}
            fetch_seq = [(p, qc) for p in range(NP) for qc in range(NQC)]
            fetched = [0]

            def fetch_upto(n):
                while fetched[0] < min(n, len(fetch_seq)):
                    p_, qc_ = fetch_seq[fetched[0]]
                    kt = kpool.tile([128, 512], BF16, name="kTc", tag="kTc")
                    nc.sync.dma_start(out=kt, in_=ktd[(p_, qc_)])
                    kch[(p_, qc_)] = kt
                    qt = qpool.tile([128, 512], BF16, name="qTc", tag="qTc")
                    nc.sync.dma_start(out=qt, in_=qtd[(p_, qc_)])
                    qch[(p_, qc_)] = qt
                    fetched[0] += 1

            # ---------------- phase 1: projections ----------------
            with (
                tc.tile_pool(name="ph1", bufs=1) as ph1,
                tc.tile_pool(name="bounce", bufs=3) as bpool,
                tc.tile_pool(name="psp", bufs=6, space="PSUM") as psp,
            ):
                wv_sb = ph1.tile([128, CT, 512], BF16)
                wq_sb = ph1.tile([128, CT, NP, 128], BF16)
                wk_sb = ph1.tile([128, CT, NP, 128], BF16)
                # one tile per (cc, tc) chunk to avoid false WAR deps
                xT = {
                    (cc, tc_): ph1.tile([128, 512], BF16, name=f"xT{cc}_{tc_}")
                    for cc in range(CT)
                    for tc_ in range(4)
                }
                cp_flip = [0]

                def copy_alt(out, in_):
                    eng = cp_flip[0] % 2
                    cp_flip[0] += 1
                    if eng == 0:
                        nc.vector.tensor_copy(out=out, in_=in_)
                    else:
                        nc.scalar.copy(out=out, in_=in_)

                def load_xt_chunk(tc_):
                    for cc in range(CT):
                        nc.sync.dma_start(
                            out=xT[(cc, tc_)],
                            in_=xat[ts(cc, 128), ts(tc_, 512)],
                        )

                # interleave DMA issue so the first consumers unblock first
                load_xt_chunk(0)
                nc.gpsimd.dma_start(out=wv_sb, in_=wvr)
                load_xt_chunk(1)
                nc.gpsimd.dma_start(out=wq_sb, in_=wqr)
                nc.gpsimd.dma_start(out=wk_sb, in_=wkr)
                load_xt_chunk(2)
                nc.gpsimd.dma_start(out=wo_sb, in_=wor)
                nc.gpsimd.dma_start(out=tr16, in_=tri)
                load_xt_chunk(3)

                def vproj_tt(tt):
                    ps = psp.tile([128, 512], F32)
                    for cc in range(CT):
                        mm(
                            ps,
                            lhsT=xT[(cc, tt // 4)][:, ts(tt % 4, 128)],
                            rhs=wv_sb[:, cc, :],
                            start=(cc == 0),
                            stop=(cc == CT - 1),
                        )
                    nc.vector.tensor_copy(
                        out=vS[:, tt, :, 0:64],
                        in_=ps.rearrange("p (h d) -> p h d", h=8),
                    )

                def qkproj_tc(tc_):
                    for p in range(NP):
                        for w_sb, dst in ((wq_sb, qtd), (wk_sb, ktd)):
                            ps = psp.tile([128, 512], F32)
                            for cc in range(CT):
                                mm(
                                    ps,
                                    lhsT=w_sb[:, cc, p, :],
                                    rhs=xT[(cc, tc_)],
                                    start=(cc == 0),
                                    stop=(cc == CT - 1),
                                )
                            bo = bpool.tile([128, 512], BF16)
                            copy_alt(bo, ps)
                            nc.scalar.dma_start(out=dst[(p, tc_)], in_=bo)

                for tc_ in range(4):
                    for tt in range(4 * tc_, 4 * tc_ + 4):
                        vproj_tt(tt)
                    qkproj_tc(tc_)
                    fetch_upto(tc_ + 1)

            # ---------------- phase 2: attention ----------------
            with (
                tc.tile_pool(name="ptp", bufs=14) as ptpool,
                tc.tile_pool(name="tmp", bufs=8) as tmppool,
                tc.tile_pool(name="rsm", bufs=6) as rpool,
                tc.tile_pool(name="rbcp", bufs=4) as rbcpool,
                tc.tile_pool(name="psS", bufs=2, space="PSUM") as psS,
                tc.tile_pool(name="psAv", bufs=4, space="PSUM") as psAv,
            ):
                avs = {}
                pts = {}
                eng_load = [0.0, 0.0]  # ns accumulated: [ACT, DVE]
                trb16 = tr16[:, None, :].to_broadcast([128, 2, 128])

                def s_exp(p, qc, j):
                    off = max(0, 128 * j - 512 * qc)
                    sg = psS.tile([128, 2, 512], F32)
                    kc = kch[(p, j // 4)]
                    qc_t = qch[(p, qc)]
                    jo = 128 * (j % 4)
                    for m in range(2):
                        mm(
                            sg[:, m, off:],
                            lhsT=kc[64 * m : 64 * m + 64, jo : jo + 128],
                            rhs=qc_t[64 * m : 64 * m + 64, off:],
                            start=True,
                            stop=True,
                        )
                    ptile = ptpool.tile([128, 2, 512], BF16)
                    masked = j >= 4 * qc
                    cols = 2 * (512 - off)
                    cost_a = cols * 1.08 + 150.0
                    cost_d = cols * 1.75 + 250.0
                    if masked:
                        cost_a += 420.0  # DVE mask-mul it will require
                    use_act = eng_load[0] + cost_a <= eng_load[1] + cost_d
                    if use_act:
                        eng_load[0] += cols * 1.08 + 150.0
                        nc.scalar.activation(
                            out=ptile[:, :, off:], in_=sg[:, :, off:],
                            func=EXP, scale=LN2,
                        )
                        if masked:
                            eng_load[1] += 420.0
                            nc.vector.tensor_mul(
                                ptile[:, :, off : off + 128],
                                ptile[:, :, off : off + 128],
                                trb16,
                            )
                    else:
                        eng_load[1] += cost_d
                        tmp = tmppool.tile([128, 2, 512], F32)
                        nc.vector._custom_dve(
                            EXP_P1, out=tmp[:, :, off:], in0=sg[:, :, off:],
                            imm2=LN2 / EXP_N,
                        )
                        if masked:
                            nc.vector._custom_dve(
                                EXP_SQ7M,
                                out=ptile[:, :, off : off + 128],
                                in0=tmp[:, :, off : off + 128],
                                in1=trb16,
                            )
                            if off + 128 < 512:
                                nc.vector._custom_dve(
                                    EXP_SQ7,
                                    out=ptile[:, :, off + 128 :],
                                    in0=tmp[:, :, off + 128 :],
                                )
                        else:
                            nc.vector._custom_dve(
                                EXP_SQ7, out=ptile[:, :, off:], in0=tmp[:, :, off:]
                            )
                    pts[(p, qc, j)] = (ptile, off)

                def av_mm(p, qc, j, nj):
                    ptile, off = pts.pop((p, qc, j))
                    av = avs[(p, qc)]
                    for m in range(2):
                        mm(
                            av[m][:65, off:],
                            lhsT=vS[:, j, 2 * p + m, :],
                            rhs=ptile[:, m, off:],
                            start=(j == 0),
                            stop=(j == nj - 1),
                        )

                def normalize(p, qc):
                    av = avs.pop((p, qc))
                    rinvs = []
                    for m in range(2):
                        rsb = rpool.tile([1, 512], F32, name="rsb", tag="rsb")
                        nc.vector.tensor_copy(out=rsb, in_=av[m][64:65, :])
                        rinv = rpool.tile([1, 512], F32, name="rinv", tag="rinv")
                        nc.vector.reciprocal_approx_fast(out=rinv, in_=rsb)
                        rinvs.append(rinv)
                    for m in range(2):
                        rb = rbcpool.tile([128, 512], F32, name="rb", tag="rb")
                        nc.gpsimd.partition_broadcast(rb, rinvs[m])
                        dst = OT[64 * m : 64 * m + 64, p, ts(qc, 512)]
                        if m == 0:
                            nc.vector.tensor_mul(dst, av[m][0:64, :], rb[0:64, :])
                            eng_load[1] += 800.0
                        else:
                            nc.vector.tensor_copy(out=dst, in_=av[m][0:64, :])
                            nc.vector.tensor_mul(dst, dst, rb[64:128, :])
                            eng_load[1] += 1600.0

                groups = []
                for p in range(NP):
                    for qc in range(NQC):
                        nj = 4 * qc + 4
                        js = list(range(nj))
                        sub = [js[i : i + 3] for i in range(0, nj, 3)]
                        for gi, jg in enumerate(sub):
                            groups.append((p, qc, nj, jg, gi == 0, gi == len(sub) - 1))

                def av_group(gi):
                    p, qc, nj, jg, first, last = groups[gi]
                    if first:
                        avs[(p, qc)] = [
                            psAv.tile([128, 512], F32, name="av", tag="av")
                            for _ in range(2)
                        ]
                    for j in jg:
                        av_mm(p, qc, j, nj)
                    if last:
                        normalize(p, qc)

                LAG = 3
                seq_pos = {(pq): i for i, pq in enumerate(fetch_seq)}
                for i in range(len(groups) + LAG):
                    if i < len(groups):
                        p, qc, nj, jg, first, last = groups[i]
                        if first:
                            fetch_upto(seq_pos[(p, qc)] + 3)
                        for j in jg:
                            s_exp(p, qc, j)
                    if i >= LAG and (i - LAG) % 2 == 1:
                        av_group(i - LAG - 1)
                        av_group(i - LAG)
                if len(groups) % 2 == 1:
                    av_group(len(groups) - 1)

            # ---------------- phase 3: output projection ----------------
            with (
                tc.tile_pool(name="obnc", bufs=4) as opool,
                tc.tile_pool(name="psO", bufs=8, space="PSUM") as psO,
            ):
                ob_flip = [0]
                for tc_ in range(4):
                    for ft in range(8):
                        pso = psO.tile([128, 512], F32, name="pso", tag="pso")
                        for p in range(NP):
                            mm(
                                pso,
                                lhsT=wo_sb[:, p, ts(ft, 128)],
                                rhs=OT[:, p, ts(tc_, 512)],
                                start=(p == 0),
                                stop=(p == NP - 1),
                            )
                        ob = opool.tile([128, 512], F32)
                        if ob_flip[0] % 2 == 0:
                            nc.vector.tensor_copy(out=ob, in_=pso)
                        else:
                            nc.scalar.copy(out=ob, in_=pso)
                        ob_flip[0] += 1
                        nc.sync.dma_start(out=ot[ts(ft, 128), ts(tc_, 512)], in_=ob)

    nc.compile()
    return nc


def kernel(x, W_qkv, b_qkv, W_out, b_out):
    global _last_in_maps
    x = np.asarray(x, dtype=np.float32)
    W_qkv = np.asarray(W_qkv, dtype=np.float32)
    b_qkv = np.asarray(b_qkv, dtype=np.float32)
    W_out = np.asarray(W_out, dtype=np.float32)
    b_out = np.asarray(b_out, dtype=np.float32)
    B = x.shape[0]

    aug = bool(np.any(b_qkv))
    CT = 9 if aug else 8
    if CT not in _CACHE:
        _CACHE[CT] = _build(CT)
    nc = _CACHE[CT]

    # triangle keep-mask for the diagonal 128 block: [p, c] = 1 if c >= p
    tri = (np.arange(128)[None, :] >= np.arange(128)[:, None]).astype(BF)

    in_maps = []
    for core in range(8):
        b, g = core // 2, core % 2
        xa = x[b]
        if aug:
            pad = np.zeros((T, 128), np.float32)
            pad[:, 0] = 1.0
            xa = np.concatenate([xa, pad], axis=1)

        def wslice(col0, qscale=1.0):
            w = W_qkv[:, col0 + 512 * g : col0 + 512 * g + 512]
            if aug:
                extra = np.zeros((128, 512), np.float32)
                extra[0] = b_qkv[col0 + 512 * g : col0 + 512 * g + 512]
                w = np.concatenate([w, extra], axis=0)
            if qscale != 1.0:
                w = w * np.float32(qscale)
            return w

        # host-side rearrange into the exact SBUF layouts, cast to bf16
        wq = wslice(0, qscale=SCALE * LOG2E)
        wk = wslice(1024)
        wv = wslice(2048)
        wqr = np.ascontiguousarray(
            wq.reshape(CT, 128, NP, 128).transpose(1, 0, 2, 3).astype(BF))
        wkr = np.ascontiguousarray(
            wk.reshape(CT, 128, NP, 128).transpose(1, 0, 2, 3).astype(BF))
        wvr = np.ascontiguousarray(
            wv.reshape(CT, 128, 512).transpose(1, 0, 2).astype(BF))
        wor = np.ascontiguousarray(
            W_out[512 * g : 512 * g + 512, :].reshape(NP, 128, 1024)
            .transpose(1, 0, 2).astype(BF))

        in_maps.append(
            {
                "xat": np.ascontiguousarray(xa.T.astype(BF)),
                "wqr": wqr,
                "wkr": wkr,
                "wvr": wvr,
                "wor": wor,
                "tri": tri,
            }
        )

    _last_in_maps = in_maps
    res = bass_utils.run_bass_kernel_spmd(nc, in_maps, list(range(8))).results
    out = np.empty((B, T, 1024), np.float32)
    for b in range(B):
        acc = res[2 * b]["ot"] + res[2 * b + 1]["ot"]
        out[b] = acc.T + b_out[None, :]
    return out
